# revision 3
# baseline (speedup 1.0000x reference)
"""ConvFrameDecoder kernel for 8 trn2 NeuronCores.

Strategy: pure data parallelism over the batch (B=128 -> 16 per core), as the
sequential t-loop recurrence is local per batch item. All parameters are
replicated; enc/frames/h0/c0 are sharded on dim 0. The per-core program is an
XLA program on each NeuronCore with:
  - the visual-encoder (1x1 convs + BN + fc) hoisted out of the t-loop and
    computed for all (b, t) in a few large matmuls (memory-bound streaming of
    `frames`, the dominant input),
  - the 25-step recurrence fully unrolled (no scan -> no neuron-hostile ops),
  - greedy argmax feedback rewritten as max + exact-compare one-hot followed by
    a one-hot @ emb_w matmul (jnp.argmax + gather does not compile on neuron).
Everything runs in fp32; this problem is numerically chaotic (greedy argmax
feedback amplifies any rounding difference), so matching reference op order in
fp32 keeps the error at the platform-divergence floor.
"""

import numpy as np

B, L, T_MAX = 128, 512, 25
DHID, DFRAME, DEMB, V = 1024, 512, 128, 512
DIN = DHID + DFRAME + DEMB
EPS = 1e-5
N_CORES = 8

_CACHE = {}


def _get_fn(T):
    import jax
    import jax.numpy as jnp

    if T in _CACHE:
        return _CACHE[T]

    def decode_shard(enc, frames, h0, c0, emb_w, go, conv1_w, conv1_b, bn1_g, bn1_b,
                     bn1_m, bn1_v, conv2_w, conv2_b, bn2_g, bn2_b, bn2_m, bn2_v,
                     fc_w, fc_b, hfc_w, hfc_b, w_ih, b_ih, w_hh, b_hh,
                     actor_w, actor_b):
        b = enc.shape[0]
        # ---- visual encoder for all (b, t) at once ----
        x = frames[:, :T].reshape(b * T, 512, 49)
        x = jnp.einsum('nck,oc->nok', x, conv1_w) + conv1_b[None, :, None]
        rs1 = jax.lax.rsqrt(bn1_v + EPS)
        x = bn1_g[None, :, None] * (x - bn1_m[None, :, None]) * rs1[None, :, None] \
            + bn1_b[None, :, None]
        x = jax.nn.relu(x)
        x = jnp.einsum('nck,oc->nok', x, conv2_w) + conv2_b[None, :, None]
        rs2 = jax.lax.rsqrt(bn2_v + EPS)
        x = bn2_g[None, :, None] * (x - bn2_m[None, :, None]) * rs2[None, :, None] \
            + bn2_b[None, :, None]
        x = jax.nn.relu(x)
        vis_all = x.reshape(b, T, 64 * 49) @ fc_w.T + fc_b  # [b, T, DFRAME]

        h, c = h0, c0
        e = jnp.broadcast_to(go, (b, DEMB))
        actions = []
        scores = []
        for t in range(T):
            q = h @ hfc_w.T + hfc_b
            raw = jnp.einsum('bld,bd->bl', enc, q)
            score = jax.nn.softmax(raw, axis=1)
            weighted = jnp.einsum('bl,bld->bd', score, enc)
            inp = jnp.concatenate([vis_all[:, t], weighted, e], axis=1)
            gates = inp @ w_ih.T + b_ih + h @ w_hh.T + b_hh
            i_, f_, g_, o_ = jnp.split(gates, 4, axis=1)
            c = jax.nn.sigmoid(f_) * c + jax.nn.sigmoid(i_) * jnp.tanh(g_)
            h = jax.nn.sigmoid(o_) * jnp.tanh(c)
            cont = jnp.concatenate([h, inp], axis=1)
            action = (cont @ actor_w.T + actor_b) @ emb_w.T  # [b, V]
            # greedy feedback without argmax/gather: exact-max one-hot matmul
            mx = jnp.max(action, axis=1, keepdims=True)
            onehot = (action == mx).astype(jnp.float32)
            onehot = onehot / jnp.sum(onehot, axis=1, keepdims=True)
            e = onehot @ emb_w
            actions.append(action)
            scores.append(score)
        actions = jnp.stack(actions, axis=1)             # [b, T, V]
        scores = jnp.stack(scores, axis=1)[..., None]    # [b, T, L, 1]
        return actions, scores, h, c

    n_params = 24
    fn = jax.pmap(
        decode_shard,
        in_axes=(0, 0, 0, 0) + (None,) * n_params,
        devices=jax.devices()[:N_CORES],
    )
    _CACHE[T] = fn
    return fn


def kernel(enc, frames, h0, c0, max_decode, emb_w, go, conv1_w, conv1_b, bn1_g,
           bn1_b, bn1_m, bn1_v, conv2_w, conv2_b, bn2_g, bn2_b, bn2_m, bn2_v,
           fc_w, fc_b, hfc_w, hfc_b, w_ih, b_ih, w_hh, b_hh, actor_w, actor_b):
    T = min(int(max_decode), int(np.asarray(frames).shape[1]))
    fn = _get_fn(T)

    f32 = np.float32
    enc = np.ascontiguousarray(np.asarray(enc, f32)).reshape(N_CORES, B // N_CORES, L, DHID)
    fr = np.asarray(frames, f32)
    fr = np.ascontiguousarray(fr).reshape(N_CORES, B // N_CORES, fr.shape[1], 512, 7, 7)
    h0 = np.ascontiguousarray(np.asarray(h0, f32)).reshape(N_CORES, B // N_CORES, DHID)
    c0 = np.ascontiguousarray(np.asarray(c0, f32)).reshape(N_CORES, B // N_CORES, DHID)

    params = [np.asarray(p, f32) for p in
              (emb_w, go, conv1_w, conv1_b, bn1_g, bn1_b, bn1_m, bn1_v,
               conv2_w, conv2_b, bn2_g, bn2_b, bn2_m, bn2_v, fc_w, fc_b,
               hfc_w, hfc_b, w_ih, b_ih, w_hh, b_hh, actor_w, actor_b)]
    actions, scores, h, c = fn(enc, fr, h0, c0, *params)

    actions = np.asarray(actions).reshape(B, T, V)
    scores = np.asarray(scores).reshape(B, T, L, 1)
    h = np.asarray(h).reshape(B, DHID)
    c = np.asarray(c).reshape(B, DHID)
    return actions, scores, h, c


# revision 5
# speedup vs baseline: 111.9716x; 111.9716x over previous
"""ConvFrameDecoder kernel for 8 trn2 NeuronCores.

Strategy: pure data parallelism over the batch (B=128 -> 16 per core), as the
sequential t-loop recurrence is local per batch item. All parameters are
replicated; enc/frames/h0/c0 are sharded on dim 0. The per-core program is an
XLA program on each NeuronCore with:
  - the visual-encoder (1x1 convs + BN + fc) hoisted out of the t-loop and
    computed for all (b, t) in a few large matmuls (memory-bound streaming of
    `frames`, the dominant input),
  - the 25-step recurrence fully unrolled (no scan -> no neuron-hostile ops),
  - greedy argmax feedback rewritten as max + exact-compare one-hot followed by
    a one-hot @ emb_w matmul (jnp.argmax + gather does not compile on neuron).
Everything runs in fp32; this problem is numerically chaotic (greedy argmax
feedback amplifies any rounding difference), so matching reference op order in
fp32 keeps the error at the platform-divergence floor.
"""

import numpy as np

B, L, T_MAX = 128, 512, 25
DHID, DFRAME, DEMB, V = 1024, 512, 128, 512
DIN = DHID + DFRAME + DEMB
EPS = 1e-5
N_CORES = 8

_CACHE = {}


def _get_fn(T):
    import jax
    import jax.numpy as jnp

    if T in _CACHE:
        return _CACHE[T]

    def decode_shard(enc, frames, h0, c0, emb_w, go, conv1_w, conv1_b, bn1_g, bn1_b,
                     bn1_m, bn1_v, conv2_w, conv2_b, bn2_g, bn2_b, bn2_m, bn2_v,
                     fc_w, fc_b, hfc_w, hfc_b, w_ih, b_ih, w_hh, b_hh,
                     actor_w, actor_b):
        b = enc.shape[0]
        # ---- visual encoder for all (b, t) at once ----
        x = frames[:, :T].reshape(b * T, 512, 49)
        x = jnp.einsum('nck,oc->nok', x, conv1_w) + conv1_b[None, :, None]
        rs1 = jax.lax.rsqrt(bn1_v + EPS)
        x = bn1_g[None, :, None] * (x - bn1_m[None, :, None]) * rs1[None, :, None] \
            + bn1_b[None, :, None]
        x = jax.nn.relu(x)
        x = jnp.einsum('nck,oc->nok', x, conv2_w) + conv2_b[None, :, None]
        rs2 = jax.lax.rsqrt(bn2_v + EPS)
        x = bn2_g[None, :, None] * (x - bn2_m[None, :, None]) * rs2[None, :, None] \
            + bn2_b[None, :, None]
        x = jax.nn.relu(x)
        vis_all = x.reshape(b, T, 64 * 49) @ fc_w.T + fc_b  # [b, T, DFRAME]

        h, c = h0, c0
        e = jnp.broadcast_to(go, (b, DEMB))
        actions = []
        scores = []
        for t in range(T):
            q = h @ hfc_w.T + hfc_b
            raw = jnp.einsum('bld,bd->bl', enc, q)
            score = jax.nn.softmax(raw, axis=1)
            weighted = jnp.einsum('bl,bld->bd', score, enc)
            inp = jnp.concatenate([vis_all[:, t], weighted, e], axis=1)
            gates = inp @ w_ih.T + b_ih + h @ w_hh.T + b_hh
            i_, f_, g_, o_ = jnp.split(gates, 4, axis=1)
            c = jax.nn.sigmoid(f_) * c + jax.nn.sigmoid(i_) * jnp.tanh(g_)
            h = jax.nn.sigmoid(o_) * jnp.tanh(c)
            cont = jnp.concatenate([h, inp], axis=1)
            action = (cont @ actor_w.T + actor_b) @ emb_w.T  # [b, V]
            # greedy feedback without argmax/gather: exact-max one-hot matmul
            mx = jnp.max(action, axis=1, keepdims=True)
            onehot = (action == mx).astype(jnp.float32)
            onehot = onehot / jnp.sum(onehot, axis=1, keepdims=True)
            e = onehot @ emb_w
            actions.append(action)
            scores.append(score)
        actions = jnp.stack(actions, axis=1)             # [b, T, V]
        scores = jnp.stack(scores, axis=1)[..., None]    # [b, T, L, 1]
        return actions, scores, h, c

    fn = jax.pmap(decode_shard, in_axes=0, devices=jax.devices()[:N_CORES])
    _CACHE[T] = fn
    return fn


def _shard(x, name=None):
    """Split on dim 0 into one device buffer per core (parallel transfers)."""
    import jax

    devs = jax.devices()[:N_CORES]
    n = x.shape[0] // N_CORES
    parts = [jax.device_put(x[i * n:(i + 1) * n], devs[i]) for i in range(N_CORES)]
    return jax.device_put_sharded(parts, devs)


def _replicate(xs):
    import jax

    devs = jax.devices()[:N_CORES]
    return [jax.device_put_replicated(x, devs) for x in xs]


def prepare(enc, frames, h0, c0, params, T):
    """Stage all inputs on the 8 cores. Returns device buffers."""
    f32 = np.float32
    enc = np.ascontiguousarray(np.asarray(enc, f32))
    fr = np.ascontiguousarray(np.asarray(frames, f32)[:, :T])
    h0 = np.ascontiguousarray(np.asarray(h0, f32))
    c0 = np.ascontiguousarray(np.asarray(c0, f32))
    dev_in = [_shard(x) for x in (enc, fr, h0, c0)]
    dev_params = _replicate([np.asarray(p, f32) for p in params])
    return dev_in, dev_params


def execute(fn, dev_in, dev_params):
    import jax

    out = fn(*dev_in, *dev_params)
    jax.block_until_ready(out)
    return out


def kernel(enc, frames, h0, c0, max_decode, emb_w, go, conv1_w, conv1_b, bn1_g,
           bn1_b, bn1_m, bn1_v, conv2_w, conv2_b, bn2_g, bn2_b, bn2_m, bn2_v,
           fc_w, fc_b, hfc_w, hfc_b, w_ih, b_ih, w_hh, b_hh, actor_w, actor_b):
    T = min(int(max_decode), int(np.asarray(frames).shape[1]))
    fn = _get_fn(T)

    params = (emb_w, go, conv1_w, conv1_b, bn1_g, bn1_b, bn1_m, bn1_v,
              conv2_w, conv2_b, bn2_g, bn2_b, bn2_m, bn2_v, fc_w, fc_b,
              hfc_w, hfc_b, w_ih, b_ih, w_hh, b_hh, actor_w, actor_b)
    dev_in, dev_params = prepare(enc, frames, h0, c0, params, T)
    actions, scores, h, c = execute(fn, dev_in, dev_params)

    actions = np.asarray(actions).reshape(B, T, V)
    scores = np.asarray(scores).reshape(B, T, L, 1)
    h = np.asarray(h).reshape(B, DHID)
    c = np.asarray(c).reshape(B, DHID)
    return actions, scores, h, c
